# revision 18
# baseline (speedup 1.0000x reference)
"""Distributed Trainium2 kernel for the AIM-policy gradient-combine problem.

Math (reference):
    gram = G @ G.T                       # [T, T], T=16, D=8388608
    norms = sqrt(diag(gram)) + 1e-8
    cos = gram / outer(norms, norms)
    w = sigmoid(10 * (tau - cos))
    coeff = w * gram / norms^2 * (1 - I)
    out = G.sum(0) - coeff.sum(0) @ G    # [D]
        = (1 - colsum(coeff)) @ G

Sharding: D axis split over 8 cores (DL = D/8 per core).  Each core:
  Phase A: local partial gram via TensorE (d on partitions, packed
           [128,128]x[128,256] float32r matmuls accumulating one PSUM tile).
  Phase B: AllReduce of the [16,16] gram, tiny on-device coefficient math
           producing wfin[j] = 1 - colsum(coeff)[j].
  Phase C: out_local = wfin @ G_local via a replicated block-diagonal
           stationary [128, 128] weight and [128, 512] rhs tiles re-read
           from HBM; outputs are evicted from 32-aligned PSUM replicas.
"""

import numpy as np

import concourse.bass as bass
import concourse.bacc as bacc
import concourse.mybir as mybir
import concourse.tile as tile
from concourse.bass_utils import run_bass_kernel_spmd

T = 16
D = 8388608
NCORES = 8
DL = D // NCORES

F32 = mybir.dt.float32
F32R = mybir.dt.float32r
AX = mybir.AxisListType
ALU = mybir.AluOpType
ACTF = mybir.ActivationFunctionType


def _host_constants():
    i16 = np.eye(T, dtype=np.float32)                      # identity [16,16]
    mask16 = (1.0 - np.eye(T)).astype(np.float32)          # zero-diagonal mask
    ones_row = np.ones((1, T), dtype=np.float32)
    ones_col = np.ones((T, 1), dtype=np.float32)
    itile = np.zeros((T, 128), dtype=np.float32)           # itile[j, k] = (k%16==j)
    for k in range(128):
        itile[k % T, k] = 1.0
    # phase-C weight mask: row k=(b*16+j), col m=(c*8+b'): 1 iff b'==b
    mask_bd = np.zeros((128, 128), dtype=np.float32)
    for b in range(8):
        for j in range(T):
            for c in range(4 * 4):
                mask_bd[b * T + j, c * 8 + b] = 1.0
    return i16, mask16, ones_row, ones_col, itile, mask_bd


def build_nc(dl=DL, pa_chunk=65536, pc_big=16384, n_cores=NCORES,
             rhs_bufs=8, cat_bufs=2, debug=False):
    """Build the SPMD bass graph for one core (same graph on all cores)."""
    assert dl % pa_chunk == 0 and pa_chunk % 128 == 0
    PA_L = pa_chunk // 128          # per-partition run length in phase A
    assert PA_L % 4 == 0
    N_CHUNK = dl // pa_chunk
    assert dl % pc_big == 0 and pc_big % 4096 == 0
    MM_PER_BIG = pc_big // 4096
    N_BIG = dl // pc_big

    nc = bacc.Bacc(trn_type="TRN2", target_bir_lowering=False,
                   num_devices=n_cores)

    g = nc.declare_dram_parameter("g", [T, dl], F32, isOutput=False)
    tau10 = nc.declare_dram_parameter("tau10", [T, T], F32, isOutput=False)
    out = nc.declare_dram_parameter("out", [dl], F32, isOutput=True)
    if debug:
        gram_dbg = nc.declare_dram_parameter("gram_dbg", [T, T], F32,
                                             isOutput=True)
        wfin_dbg = nc.declare_dram_parameter("wfin_dbg", [T, 1], F32,
                                             isOutput=True)

    (i16_np, mask16_np, ones_row_np, ones_col_np, itile_np,
     mask_bd_np) = _host_constants()
    i16_d = nc.inline_tensor(i16_np, "i16c")
    mask16_d = nc.inline_tensor(mask16_np, "mask16c")
    ones_row_d = nc.inline_tensor(ones_row_np, "onesrowc")
    ones_col_d = nc.inline_tensor(ones_col_np, "onescolc")
    itile_d = nc.inline_tensor(itile_np, "itilec")
    mask_bd_d = nc.inline_tensor(mask_bd_np, "maskbdc")

    with tile.TileContext(nc) as tc:
        with (
            tc.tile_pool(name="cat", bufs=cat_bufs) as cat_pool,
            tc.tile_pool(name="rhs", bufs=rhs_bufs) as rhs_pool,
            tc.tile_pool(name="outb", bufs=3) as out_pool,
            tc.tile_pool(name="small", bufs=1) as small_pool,
            tc.tile_pool(name="gps", bufs=1, space="PSUM") as gram_ps_pool,
            tc.tile_pool(name="cps", bufs=4, space="PSUM") as c_ps_pool,
            tc.tile_pool(name="tps", bufs=2, space="PSUM") as tiny_ps_pool,
            tc.tile_pool(name="dram", bufs=1, space="DRAM") as dram_pool,
        ):
            # ---- constants to SBUF ----
            i16_sb = small_pool.tile([T, T], F32, tag="i16")
            mask16_sb = small_pool.tile([T, T], F32, tag="mask16")
            ones_row_sb = small_pool.tile([1, T], F32, tag="onesrow")
            ones_col_sb = small_pool.tile([T, 1], F32, tag="onescol")
            itile_sb = small_pool.tile([T, 128], F32, tag="itile")
            mask_bd_sb = small_pool.tile([128, 128], F32, tag="maskbd")
            tau10_sb = small_pool.tile([T, T], F32, tag="tau10")
            nc.sync.dma_start(out=i16_sb[:], in_=i16_d[:, :])
            nc.sync.dma_start(out=mask16_sb[:], in_=mask16_d[:, :])
            nc.sync.dma_start(out=ones_row_sb[:], in_=ones_row_d[:, :])
            nc.sync.dma_start(out=ones_col_sb[:], in_=ones_col_d[:, :])
            nc.sync.dma_start(out=itile_sb[:], in_=itile_d[:, :])
            nc.sync.dma_start(out=mask_bd_sb[:], in_=mask_bd_d[:, :])
            nc.sync.dma_start(out=tau10_sb[:], in_=tau10[:, :])

            # ---- Phase A: local partial gram ----
            # One [128,16]x[128,16] matmul per l0 (columns j*PA_L+l0,
            # a single strided free dim). Rotate over 4 PE col-groups via
            # tile_position so up to 4 matmuls run concurrently; each
            # col-group accumulates its own 32-aligned PSUM region.
            gram_ps = gram_ps_pool.tile([128, T], F32, tag="gramps")
            n_mm = N_CHUNK * PA_L
            mm_idx = 0
            for c in range(N_CHUNK):
                cat = cat_pool.tile([128, T * PA_L], F32, tag="cat")
                src = g[:, c * pa_chunk:(c + 1) * pa_chunk] \
                    .rearrange("j (p l) -> p j l", p=128, l=PA_L)
                dst = cat[:].rearrange("p (j l) -> p j l", j=T, l=PA_L)
                nc.gpsimd.dma_start(out=dst, in_=src)
                cat_v = cat[:].rearrange("p (j l) -> p l j", j=T, l=PA_L)
                for l0 in range(PA_L):
                    cg = mm_idx % 4
                    op = cat_v[:, l0]
                    nc.tensor.matmul(
                        gram_ps[32 * cg:32 * cg + T, :], op, op,
                        start=(mm_idx < 4), stop=(mm_idx >= n_mm - 4),
                        tile_position=(0, 32 * cg),
                        skip_group_check=True)
                    mm_idx += 1

            # ---- extract local gram: sum the 4 col-group regions ----
            acc_a = small_pool.tile([T, T], F32, tag="acca")
            acc_b = small_pool.tile([T, T], F32, tag="accb")
            nc.vector.tensor_copy(acc_a[:], gram_ps[0:T, :])
            cur, nxt = acc_a, acc_b
            for cg in range(1, 4):
                nc.vector.tensor_tensor(nxt[:], cur[:],
                                        gram_ps[32 * cg:32 * cg + T, :],
                                        op=ALU.add)
                cur, nxt = nxt, cur
            gram_loc = cur

            # ---- Phase B: allreduce + coefficient math ----
            cc_in = dram_pool.tile([T, T], F32, tag="ccin")
            cc_out = dram_pool.tile([T, T], F32, tag="ccout")
            nc.sync.dma_start(out=cc_in[:], in_=gram_loc[:])
            nc.gpsimd.collective_compute(
                "AllReduce",
                ALU.add,
                replica_groups=[list(range(n_cores))],
                ins=[cc_in.opt()],
                outs=[cc_out.opt()],
            )
            gram_sb = small_pool.tile([T, T], F32, tag="gram")
            nc.sync.dma_start(out=gram_sb[:], in_=cc_out[:])

            # diag -> dvec [16,1]
            tmp16 = small_pool.tile([T, T], F32, tag="tmp16")
            dvec = small_pool.tile([T, 1], F32, tag="dvec")
            nc.vector.tensor_tensor(tmp16[:], gram_sb[:], i16_sb[:], op=ALU.mult)
            nc.vector.reduce_sum(dvec[:], tmp16[:], axis=AX.X)
            # inv_d = 1/diag ; norms = sqrt(diag); inv_n = 1/norms
            inv_d = small_pool.tile([T, 1], F32, tag="invd")
            nrm = small_pool.tile([T, 1], F32, tag="nrm")
            inv_n = small_pool.tile([T, 1], F32, tag="invn")
            nc.vector.reciprocal(inv_d[:], dvec[:])
            nc.scalar.sqrt(nrm[:], dvec[:])
            nc.vector.reciprocal(inv_n[:], nrm[:])

            # row-broadcast of inv_n: bc[i,j] = inv_n[j]
            tp_ps = tiny_ps_pool.tile([1, T], F32, tag="tinyps")
            nc.tensor.transpose(tp_ps[:], inv_n[:], i16_sb[:])
            row_sb = small_pool.tile([1, T], F32, tag="rowsb")
            nc.vector.tensor_copy(row_sb[:], tp_ps[:])
            bc_ps = tiny_ps_pool.tile([T, T], F32, tag="tinyps")
            nc.tensor.matmul(bc_ps[:], ones_row_sb[:], row_sb[:],
                             start=True, stop=True)
            bc_sb = small_pool.tile([T, T], F32, tag="bcsb")
            nc.vector.tensor_copy(bc_sb[:], bc_ps[:])

            # cos10 = gram * (10*inv_n[i]) * inv_n[j]
            f_i = small_pool.tile([T, 1], F32, tag="fi")
            nc.vector.tensor_scalar_mul(f_i[:], inv_n[:], 10.0)
            cosA = small_pool.tile([T, T], F32, tag="cosA")
            nc.vector.tensor_scalar_mul(cosA[:], gram_sb[:], f_i[:])
            cos10 = small_pool.tile([T, T], F32, tag="cos10")
            nc.vector.tensor_tensor(cos10[:], cosA[:], bc_sb[:], op=ALU.mult)
            # sig_in = tau10 - cos10 ; w = sigmoid(sig_in)
            sig_in = small_pool.tile([T, T], F32, tag="sigin")
            nc.vector.tensor_tensor(sig_in[:], tau10_sb[:], cos10[:],
                                    op=ALU.subtract)
            wmat = small_pool.tile([T, T], F32, tag="wmat")
            nc.scalar.activation(wmat[:], sig_in[:], ACTF.Sigmoid)
            # m1 = w * gram * mask16 ; colsum via matmul with ones_col
            m1a = small_pool.tile([T, T], F32, tag="m1a")
            m1 = small_pool.tile([T, T], F32, tag="m1")
            nc.vector.tensor_tensor(m1a[:], wmat[:], gram_sb[:], op=ALU.mult)
            nc.vector.tensor_tensor(m1[:], m1a[:], mask16_sb[:], op=ALU.mult)
            cs_ps = tiny_ps_pool.tile([T, 1], F32, tag="tinyps")
            nc.tensor.matmul(cs_ps[:], m1[:], ones_col_sb[:],
                             start=True, stop=True)
            epsp = small_pool.tile([T, 1], F32, tag="epsp")
            nc.vector.tensor_copy(epsp[:], cs_ps[:])
            eps = small_pool.tile([T, 1], F32, tag="eps")
            nc.vector.tensor_tensor(eps[:], epsp[:], inv_d[:], op=ALU.mult)
            # wfin = 1 - eps
            wfin = small_pool.tile([T, 1], F32, tag="wfin")
            nc.vector.tensor_scalar(wfin[:], eps[:], -1.0, 1.0,
                                    op0=ALU.mult, op1=ALU.add)
            # W [128, 128]: W[b*16+j, c*8+b'] = wfin[j] * (b'==b)
            w128_ps = tiny_ps_pool.tile([128, 1], F32, tag="tinyps")
            nc.tensor.matmul(w128_ps[:], itile_sb[:], wfin[:],
                             start=True, stop=True)
            w128_sb = small_pool.tile([128, 1], F32, tag="w128sb")
            nc.vector.tensor_copy(w128_sb[:], w128_ps[:])
            W_sb = small_pool.tile([128, 128], F32, tag="Wsb")
            nc.vector.tensor_scalar_mul(W_sb[:], mask_bd_sb[:], w128_sb[:])

            if debug:
                nc.sync.dma_start(out=gram_dbg[:, :], in_=gram_sb[:])
                nc.sync.dma_start(out=wfin_dbg[:, :], in_=wfin[:])

            # ---- Phase C: out = wfin @ G ----
            gmm = 0             # global mm index
            outb = None
            BM = MM_PER_BIG * 512       # contiguous d per (b, j) partition run
            for big in range(N_BIG):
                d0 = big * pc_big
                rhsb = rhs_pool.tile([128, BM], F32, tag="rhsb")
                src = g[:, d0:d0 + pc_big].rearrange(
                    "j (b m) -> b j m", b=8, m=BM)
                nc.sync.dma_start(out=rhsb[:], in_=src)
                # d-mapping: d = d0 + b*BM + k*512 + n
                dview = out[d0:d0 + pc_big].rearrange(
                    "(b k n) -> k b n", b=8, k=MM_PER_BIG, n=512)
                for k in range(MM_PER_BIG):
                    c = gmm % 4
                    if c == 0:
                        outb = out_pool.tile([128, 512], F32, tag="outb")
                    ps = c_ps_pool.tile([128, 512], F32, tag="cps")
                    nc.tensor.matmul(
                        ps[:], W_sb[:],
                        rhsb[:, k * 512:(k + 1) * 512],
                        start=True, stop=True)
                    src_ev = ps[32 * c:32 * c + 8, :]
                    dst_ev = outb[32 * c:32 * c + 8, :]
                    if k % 2 == 0:
                        nc.vector.tensor_copy(dst_ev, src_ev)
                    else:
                        nc.scalar.copy(dst_ev, src_ev)
                    nc.sync.dma_start(out=dview[k],
                                      in_=outb[32 * c:32 * c + 8, :])
                    gmm += 1

    nc.compile()
    return nc


def _shard_inputs(grads_stack, tau):
    tau10 = (10.0 * np.asarray(tau)).astype(np.float32)
    gs = np.asarray(grads_stack)
    in_maps = []
    for c in range(NCORES):
        gshard = np.ascontiguousarray(gs[:, c * DL:(c + 1) * DL],
                                      dtype=np.float32)
        in_maps.append({"g": gshard, "tau10": tau10})
    return in_maps


def kernel(grads_stack, tau):
    nc = build_nc()
    in_maps = _shard_inputs(grads_stack, tau)
    res = run_bass_kernel_spmd(nc, in_maps, list(range(NCORES)))
    outs = [res.results[c]["out"] for c in range(NCORES)]
    return np.concatenate(outs).astype(np.float32)


# revision 19
# speedup vs baseline: 1.1120x; 1.1120x over previous
"""Distributed Trainium2 kernel for the AIM-policy gradient-combine problem.

Math (reference):
    gram = G @ G.T                       # [T, T], T=16, D=8388608
    norms = sqrt(diag(gram)) + 1e-8
    cos = gram / outer(norms, norms)
    w = sigmoid(10 * (tau - cos))
    coeff = w * gram / norms^2 * (1 - I)
    out = G.sum(0) - coeff.sum(0) @ G    # [D]
        = (1 - colsum(coeff)) @ G

Sharding: D axis split over 8 cores (DL = D/8 per core).  Each core:
  Phase A: local partial gram via TensorE.  G is staged d-on-partitions in
           "super tiles" holding NQ sub-chunks (bf16, cast on ScalarE), so
           one [128,128]x[128,128] matmul contracts 128 partitions x NQ
           sub-chunks = 1024 d at once.  Column packing (q,j) uses a single
           strided free dim (stride L), which the BIR verifier allows.
  Phase B: AllReduce of the [16,16] gram (pre-warmed), coefficient math
           producing wfin[j] = 1 - colsum(coeff)[j].
  Phase C: out_local = wfin @ G_local via a replicated block-diagonal
           stationary [128, 128] weight and [128, 512] fp32 rhs tiles
           re-read from HBM; outputs evicted from 32-aligned PSUM replicas.
"""

import numpy as np

import concourse.bass as bass
import concourse.bacc as bacc
import concourse.mybir as mybir
import concourse.tile as tile
from concourse.bass_utils import run_bass_kernel_spmd

T = 16
D = 8388608
NCORES = 8
DL = D // NCORES

F32 = mybir.dt.float32
BF16 = mybir.dt.bfloat16
AX = mybir.AxisListType
ALU = mybir.AluOpType
ACTF = mybir.ActivationFunctionType

NQ = 8          # sub-chunks per super tile (packed into matmul columns)


def _host_constants():
    i16 = np.eye(T, dtype=np.float32)                      # identity [16,16]
    mask16 = (1.0 - np.eye(T)).astype(np.float32)          # zero-diagonal mask
    ones_row = np.ones((1, T), dtype=np.float32)
    ones_col = np.ones((T, 1), dtype=np.float32)
    itile = np.zeros((T, 128), dtype=np.float32)           # itile[j, k] = (k%16==j)
    for k in range(128):
        itile[k % T, k] = 1.0
    # phase-C weight mask: row k=(b*16+j), col m=(c*8+b'): 1 iff b'==b
    mask_bd = np.zeros((128, 128), dtype=np.float32)
    for b in range(8):
        for j in range(T):
            for c in range(16):
                mask_bd[b * T + j, c * 8 + b] = 1.0
    # gram psum mask: row (q*16+i), col (q'*16+j): 1 iff q'==q
    gmask = np.zeros((128, 128), dtype=np.float32)
    for q in range(NQ):
        gmask[q * T:(q + 1) * T, q * T:(q + 1) * T] = 1.0
    return i16, mask16, ones_row, ones_col, itile, mask_bd, gmask


def build_nc(dl=DL, pa_l=128, pc_big=16384, n_cores=NCORES,
             rhs_bufs=4, debug=False):
    """Build the SPMD bass graph for one core (same graph on all cores)."""
    L = pa_l                          # per-(partition,row) run length, f32 elems
    SUB = 128 * L                     # d per sub-chunk
    SUPER = NQ * SUB                  # d per super tile
    assert dl % SUPER == 0
    N_SUPER = dl // SUPER
    assert dl % pc_big == 0 and pc_big % 4096 == 0
    MM_PER_BIG = pc_big // 4096
    N_BIG = dl // pc_big

    nc = bacc.Bacc(trn_type="TRN2", target_bir_lowering=False,
                   num_devices=n_cores)

    g = nc.declare_dram_parameter("g", [T, dl], F32, isOutput=False)
    tau10 = nc.declare_dram_parameter("tau10", [T, T], F32, isOutput=False)
    out = nc.declare_dram_parameter("out", [dl], F32, isOutput=True)
    if debug:
        gram_dbg = nc.declare_dram_parameter("gram_dbg", [T, T], F32,
                                             isOutput=True)
        wfin_dbg = nc.declare_dram_parameter("wfin_dbg", [T, 1], F32,
                                             isOutput=True)

    (i16_np, mask16_np, ones_row_np, ones_col_np, itile_np,
     mask_bd_np, gmask_np) = _host_constants()
    i16_d = nc.inline_tensor(i16_np, "i16c")
    mask16_d = nc.inline_tensor(mask16_np, "mask16c")
    ones_row_d = nc.inline_tensor(ones_row_np, "onesrowc")
    ones_col_d = nc.inline_tensor(ones_col_np, "onescolc")
    itile_d = nc.inline_tensor(itile_np, "itilec")
    etile_d = nc.inline_tensor(itile_np.T.copy(), "etilec")
    mask_bd_d = nc.inline_tensor(mask_bd_np, "maskbdc")
    gmask_d = nc.inline_tensor(gmask_np, "gmaskc")

    dmae = [nc.sync, nc.scalar]       # alternate the two HWDGE rings

    with tile.TileContext(nc) as tc:
        with (
            tc.tile_pool(name="sub", bufs=3) as sub_pool,
            tc.tile_pool(name="sup", bufs=2) as sup_pool,
            tc.tile_pool(name="rhs", bufs=rhs_bufs) as rhs_pool,
            tc.tile_pool(name="outb", bufs=3) as out_pool,
            tc.tile_pool(name="small", bufs=1) as small_pool,
            tc.tile_pool(name="gps", bufs=1, space="PSUM") as gram_ps_pool,
            tc.tile_pool(name="cps", bufs=4, space="PSUM") as c_ps_pool,
            tc.tile_pool(name="tps", bufs=2, space="PSUM") as tiny_ps_pool,
            tc.tile_pool(name="dram", bufs=1, space="DRAM") as dram_pool,
        ):
            # ---- constants to SBUF ----
            i16_sb = small_pool.tile([T, T], F32, tag="i16")
            mask16_sb = small_pool.tile([T, T], F32, tag="mask16")
            ones_row_sb = small_pool.tile([1, T], F32, tag="onesrow")
            ones_col_sb = small_pool.tile([T, 1], F32, tag="onescol")
            itile_sb = small_pool.tile([T, 128], F32, tag="itile")
            etile_sb = small_pool.tile([128, T], F32, tag="etile")
            mask_bd_sb = small_pool.tile([128, 128], F32, tag="maskbd")
            gmask_sb = small_pool.tile([128, 128], F32, tag="gmask")
            tau10_sb = small_pool.tile([T, T], F32, tag="tau10")
            nc.sync.dma_start(out=i16_sb[:], in_=i16_d[:, :])
            nc.sync.dma_start(out=mask16_sb[:], in_=mask16_d[:, :])
            nc.sync.dma_start(out=ones_row_sb[:], in_=ones_row_d[:, :])
            nc.sync.dma_start(out=ones_col_sb[:], in_=ones_col_d[:, :])
            nc.sync.dma_start(out=itile_sb[:], in_=itile_d[:, :])
            nc.sync.dma_start(out=etile_sb[:], in_=etile_d[:, :])
            nc.sync.dma_start(out=mask_bd_sb[:], in_=mask_bd_d[:, :])
            nc.sync.dma_start(out=gmask_sb[:], in_=gmask_d[:, :])
            nc.sync.dma_start(out=tau10_sb[:], in_=tau10[:, :])

            # ---- collective pre-warm (overlaps phase A) ----
            warm_in = dram_pool.tile([T, T], F32, tag="warmin")
            warm_out = dram_pool.tile([T, T], F32, tag="warmout")
            nc.sync.dma_start(out=warm_in[:], in_=i16_sb[:])
            nc.gpsimd.collective_compute(
                "AllReduce", ALU.add,
                replica_groups=[list(range(n_cores))],
                ins=[warm_in.opt()], outs=[warm_out.opt()])

            # ---- Phase A: local partial gram (bf16 packed matmuls) ----
            # super tile free layout: f = q*(16L) + j*L + l  (bf16)
            # matmul l0: operand = super[:, l0::L] -> 128 cols (q,j), one
            # strided free dim.  out[128,128] accumulates in PSUM; diagonal
            # q-blocks hold the gram partials.
            gram_ps = gram_ps_pool.tile([128, 128], F32, tag="gramps")
            n_mm = N_SUPER * L
            mm_idx = 0
            for s in range(N_SUPER):
                sup = sup_pool.tile([128, T * NQ * L], BF16, tag="sup")
                for q in range(NQ):
                    d0 = s * SUPER + q * SUB
                    sub = sub_pool.tile([128, T * L], F32, tag="sub")
                    src = g[:, d0:d0 + SUB].rearrange(
                        "j (p l) -> p j l", p=128, l=L)
                    dstv = sub[:].rearrange("p (j l) -> p j l", j=T, l=L)
                    dmae[(s * NQ + q) % 2].dma_start(out=dstv, in_=src)
                    nc.scalar.copy(sup[:, q * T * L:(q + 1) * T * L], sub[:])
                sup_v = sup[:].rearrange("p (c l) -> p l c", c=T * NQ, l=L)
                for l0 in range(L):
                    op = sup_v[:, l0]
                    nc.tensor.matmul(
                        gram_ps[:], op, op,
                        start=(mm_idx == 0), stop=(mm_idx == n_mm - 1))
                    mm_idx += 1

            # ---- extract local gram: mask, reduce over q', fold over q ----
            s_full = small_pool.tile([128, 128], F32, tag="sfull")
            nc.vector.tensor_copy(s_full[:], gram_ps[:])
            sm = small_pool.tile([128, 128], F32, tag="smasked")
            nc.vector.tensor_tensor(sm[:], s_full[:], gmask_sb[:], op=ALU.mult)
            red = small_pool.tile([128, T], F32, tag="red")
            sm_v = sm[:].rearrange("p (q j) -> p j q", q=NQ, j=T)
            nc.vector.tensor_reduce(red[:], sm_v, axis=AX.X, op=ALU.add)
            fold_ps = tiny_ps_pool.tile([T, T], F32, tag="tinyps")
            nc.tensor.matmul(fold_ps[:], etile_sb[:], red[:],
                             start=True, stop=True)
            gram_loc = small_pool.tile([T, T], F32, tag="gramloc")
            nc.vector.tensor_copy(gram_loc[:], fold_ps[:])

            # ---- Phase B: allreduce + coefficient math ----
            cc_in = dram_pool.tile([T, T], F32, tag="ccin")
            cc_out = dram_pool.tile([T, T], F32, tag="ccout")
            nc.sync.dma_start(out=cc_in[:], in_=gram_loc[:])
            nc.gpsimd.collective_compute(
                "AllReduce", ALU.add,
                replica_groups=[list(range(n_cores))],
                ins=[cc_in.opt()], outs=[cc_out.opt()])
            gram_sb = small_pool.tile([T, T], F32, tag="gram")
            nc.sync.dma_start(out=gram_sb[:], in_=cc_out[:])

            # diag -> dvec [16,1]
            tmp16 = small_pool.tile([T, T], F32, tag="tmp16")
            dvec = small_pool.tile([T, 1], F32, tag="dvec")
            nc.vector.tensor_tensor(tmp16[:], gram_sb[:], i16_sb[:], op=ALU.mult)
            nc.vector.reduce_sum(dvec[:], tmp16[:], axis=AX.X)
            inv_d = small_pool.tile([T, 1], F32, tag="invd")
            nrm = small_pool.tile([T, 1], F32, tag="nrm")
            inv_n = small_pool.tile([T, 1], F32, tag="invn")
            nc.vector.reciprocal(inv_d[:], dvec[:])
            nc.scalar.sqrt(nrm[:], dvec[:])
            nc.vector.reciprocal(inv_n[:], nrm[:])

            # row-broadcast of inv_n: bc[i,j] = inv_n[j]
            tp_ps = tiny_ps_pool.tile([1, T], F32, tag="tinyps")
            nc.tensor.transpose(tp_ps[:], inv_n[:], i16_sb[:])
            row_sb = small_pool.tile([1, T], F32, tag="rowsb")
            nc.vector.tensor_copy(row_sb[:], tp_ps[:])
            bc_ps = tiny_ps_pool.tile([T, T], F32, tag="tinyps")
            nc.tensor.matmul(bc_ps[:], ones_row_sb[:], row_sb[:],
                             start=True, stop=True)
            bc_sb = small_pool.tile([T, T], F32, tag="bcsb")
            nc.vector.tensor_copy(bc_sb[:], bc_ps[:])

            # cos10 = gram * (10*inv_n[i]) * inv_n[j]
            f_i = small_pool.tile([T, 1], F32, tag="fi")
            nc.vector.tensor_scalar_mul(f_i[:], inv_n[:], 10.0)
            cosA = small_pool.tile([T, T], F32, tag="cosA")
            nc.vector.tensor_scalar_mul(cosA[:], gram_sb[:], f_i[:])
            cos10 = small_pool.tile([T, T], F32, tag="cos10")
            nc.vector.tensor_tensor(cos10[:], cosA[:], bc_sb[:], op=ALU.mult)
            sig_in = small_pool.tile([T, T], F32, tag="sigin")
            nc.vector.tensor_tensor(sig_in[:], tau10_sb[:], cos10[:],
                                    op=ALU.subtract)
            wmat = small_pool.tile([T, T], F32, tag="wmat")
            nc.scalar.activation(wmat[:], sig_in[:], ACTF.Sigmoid)
            m1a = small_pool.tile([T, T], F32, tag="m1a")
            m1 = small_pool.tile([T, T], F32, tag="m1")
            nc.vector.tensor_tensor(m1a[:], wmat[:], gram_sb[:], op=ALU.mult)
            nc.vector.tensor_tensor(m1[:], m1a[:], mask16_sb[:], op=ALU.mult)
            cs_ps = tiny_ps_pool.tile([T, 1], F32, tag="tinyps")
            nc.tensor.matmul(cs_ps[:], m1[:], ones_col_sb[:],
                             start=True, stop=True)
            epsp = small_pool.tile([T, 1], F32, tag="epsp")
            nc.vector.tensor_copy(epsp[:], cs_ps[:])
            eps = small_pool.tile([T, 1], F32, tag="eps")
            nc.vector.tensor_tensor(eps[:], epsp[:], inv_d[:], op=ALU.mult)
            wfin = small_pool.tile([T, 1], F32, tag="wfin")
            nc.vector.tensor_scalar(wfin[:], eps[:], -1.0, 1.0,
                                    op0=ALU.mult, op1=ALU.add)
            # W [128, 128]: W[b*16+j, c*8+b'] = wfin[j] * (b'==b)
            w128_ps = tiny_ps_pool.tile([128, 1], F32, tag="tinyps")
            nc.tensor.matmul(w128_ps[:], itile_sb[:], wfin[:],
                             start=True, stop=True)
            w128_sb = small_pool.tile([128, 1], F32, tag="w128sb")
            nc.vector.tensor_copy(w128_sb[:], w128_ps[:])
            W_sb = small_pool.tile([128, 128], F32, tag="Wsb")
            nc.vector.tensor_scalar_mul(W_sb[:], mask_bd_sb[:], w128_sb[:])

            if debug:
                nc.sync.dma_start(out=gram_dbg[:, :], in_=gram_sb[:])
                nc.sync.dma_start(out=wfin_dbg[:, :], in_=wfin[:])

            # ---- Phase C: out = wfin @ G ----
            gmm = 0
            outb = None
            BM = MM_PER_BIG * 512     # contiguous d per (b, j) partition run
            for big in range(N_BIG):
                d0 = big * pc_big
                rhsb = rhs_pool.tile([128, BM], F32, tag="rhsb")
                src = g[:, d0:d0 + pc_big].rearrange(
                    "j (b m) -> b j m", b=8, m=BM)
                dmae[big % 2].dma_start(out=rhsb[:], in_=src)
                # d-mapping: d = d0 + b*BM + k*512 + n
                dview = out[d0:d0 + pc_big].rearrange(
                    "(b k n) -> k b n", b=8, k=MM_PER_BIG, n=512)
                for k in range(MM_PER_BIG):
                    c = gmm % 4
                    if c == 0:
                        outb = out_pool.tile([128, 512], F32, tag="outb")
                    ps = c_ps_pool.tile([128, 512], F32, tag="cps")
                    nc.tensor.matmul(
                        ps[:], W_sb[:],
                        rhsb[:, k * 512:(k + 1) * 512],
                        start=True, stop=True)
                    src_ev = ps[32 * c:32 * c + 8, :]
                    dst_ev = outb[32 * c:32 * c + 8, :]
                    if k % 2 == 0:
                        nc.vector.tensor_copy(dst_ev, src_ev)
                    else:
                        nc.scalar.copy(dst_ev, src_ev)
                    dmae[(gmm + 1) % 2].dma_start(
                        out=dview[k], in_=outb[32 * c:32 * c + 8, :])
                    gmm += 1

    nc.compile()
    return nc


def _shard_inputs(grads_stack, tau):
    tau10 = (10.0 * np.asarray(tau)).astype(np.float32)
    gs = np.asarray(grads_stack)
    in_maps = []
    for c in range(NCORES):
        gshard = np.ascontiguousarray(gs[:, c * DL:(c + 1) * DL],
                                      dtype=np.float32)
        in_maps.append({"g": gshard, "tau10": tau10})
    return in_maps


def kernel(grads_stack, tau):
    nc = build_nc()
    in_maps = _shard_inputs(grads_stack, tau)
    res = run_bass_kernel_spmd(nc, in_maps, list(range(NCORES)))
    outs = [np.asarray(res.results[c]["out"]).ravel() for c in range(NCORES)]
    return np.concatenate(outs).astype(np.float32)
